# revision 1
# baseline (speedup 1.0000x reference)
"""GATConv Trainium2 kernel — 8-core SPMD, dst-sharded.

Sharding: dst nodes partitioned across 8 cores (12.5K each), so every
core owns all edges into its nodes and no collectives are needed; the
fp16 node-feature table is replicated per core for src gathers.

Per core, per src-chunk pass (int16 gather indices address <=32768-row
table chunks): edges grouped by exact dst-degree class D, node-major,
so each node's edges form one fixed-length run. dma_gather (transpose
mode) brings x[src]^T tiles; the PE computes h_e = W^T x and the
attention logits (replicated-Ws matmul + a class-constant staircase
matmul that adds d_dst inside PSUM); ACT applies LeakyReLU and Exp
(segment-max subtraction is skipped — logits are O(1) so exp cannot
overflow and softmax is shift-invariant); DVE forms exp*h_e and
segment-sums each D-run with one strided tensor_reduce. Per-pass
results land in position-space DRAM tables (rows = [agg|z]) via a PE
transpose; a final on-device merge gathers the 4 tables per node, sums,
normalizes by z and adds bias.
"""

import numpy as np

N = 100000
E = 1600000
IN_CH = 128
HEADS = 4
OUT_CH = 32
NEG_SLOPE = 0.2
NCORES = 8
NPC = N // NCORES            # nodes per core = 12500
CHUNK = 32767                # real rows per table chunk (row 32767 = zeros)
NCHUNK = 4
SB_SLOTS = 4096              # max slots per superblock (one gather call)
MAX_D = 32
TROW = 256                   # table row: [agg 128 | z 4 | pad] fp16
P_OUT = 12800                # padded output rows (100 blocks of 128)
MQR = 2560                   # merge round rows (5 rounds)


def _class_layout(D):
    npb = 128 if D <= 4 else (64 if D <= 8 else 32)
    return npb, D * npb


def _host_prep(x, edge_index, weight, att, bias):
    x = np.asarray(x, np.float32)
    ei = np.asarray(edge_index)
    src = ei[0].astype(np.int64)
    dst = ei[1].astype(np.int64)
    weight = np.asarray(weight, np.float32)
    att = np.asarray(att, np.float32)
    bias = np.asarray(bias, np.float32)

    # ---- gather table: 4 chunks x 32768 rows (last row of each = zeros) ----
    tbl = np.zeros((NCHUNK * (CHUNK + 1), IN_CH), np.float16)
    for g in range(NCHUNK):
        lo, hi = g * CHUNK, min((g + 1) * CHUNK, N)
        if lo < N:
            tbl[g * (CHUNK + 1): g * (CHUNK + 1) + (hi - lo)] = x[lo:hi].astype(np.float16)
    src_chunk = (src // CHUNK).astype(np.int32)
    src_local = (src % CHUNK).astype(np.int32)
    core = (dst // NPC).astype(np.int32)
    dstl = (dst % NPC).astype(np.int32)

    flat = (core.astype(np.int64) * NCHUNK + src_chunk) * NPC + dstl
    counts = np.bincount(flat, minlength=NCORES * NCHUNK * NPC).reshape(
        NCORES, NCHUNK, NPC).astype(np.int32)
    maxd = int(counts.max())
    assert maxd <= MAX_D, f"per-pass degree {maxd} > {MAX_D}"

    nclass = np.zeros((NCORES, NCHUNK, MAX_D + 1), np.int64)
    for c in range(NCORES):
        for g in range(NCHUNK):
            nclass[c, g] = np.bincount(counts[c, g], minlength=MAX_D + 1)
    uni = nclass.max(axis=0)

    # ---- uniform (cross-core) layout: per pass a list of superblocks ----
    passes, pos_total, slot_total = [], [], []
    for g in range(NCHUNK):
        sbs, pos, slot = [], 0, 0
        for D in range(1, MAX_D + 1):
            nn = int(uni[g, D])
            if nn == 0:
                continue
            npb, colsb = _class_layout(D)
            margin = (128 // npb) - 1
            max_nodes = max((SB_SLOTS // colsb - margin) * npb, npb)
            done = 0
            while done < nn:
                take = min(max_nodes, nn - done)
                take_pad = -(-take // npb) * npb
                if done + take >= nn:
                    # last sb of the class: pad positions to a 128 boundary
                    take_pad += (-(pos + take_pad)) % 128
                nslots = take_pad * D
                sbs.append(dict(D=D, npb=npb, colsb=colsb, nodes=take_pad,
                                real_nodes=take, pos0=pos, slot0=slot,
                                slots=nslots, slots_pad=-(-nslots // 128) * 128))
                pos += take_pad
                slot += -(-nslots // 128) * 128
                done += take
        passes.append(sbs)
        assert pos % 128 == 0
        pos_total.append(pos)
        slot_total.append(max(slot, 128))
    PT_MAX = -(-(max(pos_total) + 128) // 2048) * 2048

    def wrap16(a):
        S = len(a)
        w = np.empty((128, S // 16), np.int16)
        t = a.astype(np.int16).reshape(S // 16, 16).T
        for r in range(8):
            w[16 * r:16 * r + 16] = t
        return np.ascontiguousarray(w)

    ins_per_core = []
    for c in range(NCORES):
        ci = {}
        for g in range(NCHUNK):
            sbs = passes[g]
            cnts = counts[c, g]
            idx_stream = np.full(slot_total[g], CHUNK, np.int32)  # default zero-row
            posmap = np.full(NPC, pos_total[g], np.int64)          # default zero-pos
            xo_pos = np.zeros((PT_MAX,), np.int64)                 # node id per position
            xo_valid = np.zeros((PT_MAX,), bool)
            m = (core == c) & (src_chunk == g)
            e_dst = dstl[m]
            e_srcl = src_local[m]
            o = np.argsort(e_dst, kind="stable")
            e_dst, e_srcl = e_dst[o], e_srcl[o]
            starts = np.searchsorted(e_dst, np.arange(NPC))
            byD = {D: np.nonzero(cnts == D)[0] for D in range(1, MAX_D + 1)}
            ptr = {D: 0 for D in byD}
            for sb in sbs:
                D = sb["D"]
                nodes_D = byD.get(D)
                p = ptr[D]
                take = min(sb["real_nodes"], max(0, len(nodes_D) - p))
                sel = nodes_D[p:p + take]
                ptr[D] = p + take
                if take:
                    posmap[sel] = sb["pos0"] + np.arange(take)
                    xo_pos[sb["pos0"]:sb["pos0"] + take] = sel
                    xo_valid[sb["pos0"]:sb["pos0"] + take] = True
                    st = starts[sel]
                    eidx = (st[:, None] + np.arange(D)[None, :]).reshape(-1)
                    idx_stream[sb["slot0"]:sb["slot0"] + take * D] = e_srcl[eidx]
            ci[f"idx{g}"] = wrap16(idx_stream)
            mi = np.full(P_OUT, pos_total[g], np.int64)
            mi[:NPC] = posmap
            ci[f"midx{g}"] = wrap16(mi)
            # x_own^T in pass-g position order (dummy positions -> zeros)
            xo = np.zeros((PT_MAX, IN_CH), np.float16)
            ids = xo_pos[xo_valid]
            xo[np.nonzero(xo_valid)[0]] = x[c * NPC + ids].astype(np.float16)
            ci[f"xoT{g}"] = np.ascontiguousarray(xo.T)
        ins_per_core.append(ci)

    att_src = att[0, :, :OUT_CH]
    att_dst = att[0, :, OUT_CH:]
    as_bd = np.zeros((IN_CH, HEADS), np.float16)
    ad_bd = np.zeros((IN_CH, HEADS), np.float16)
    for h in range(HEADS):
        as_bd[32 * h:32 * h + 32, h] = att_src[h].astype(np.float16)
        ad_bd[32 * h:32 * h + 32, h] = att_dst[h].astype(np.float16)

    used_D = sorted({sb["D"] for sbs in passes for sb in sbs})
    st_cols, st_list, off = {}, [], 0
    for D in used_D:
        npb, colsb = _class_layout(D)
        pp = np.arange(128) % npb
        kk = np.arange(colsb) // D
        st_list.append((kk[None, :] == pp[:, None]).astype(np.float16))
        st_cols[D] = (off, colsb)
        off += colsb
    shared = {
        "tbl": tbl,
        "wT": np.ascontiguousarray(weight.T).astype(np.float16),
        "wl": weight.astype(np.float16),
        "as_bd": as_bd, "ad_bd": ad_bd,
        "biasr": np.tile(bias.astype(np.float32)[None, :], (128, 1)),
        "ident": np.eye(128, dtype=np.float16),
        "stcat": np.concatenate(st_list, axis=1),
    }
    meta = dict(passes=passes, pos_total=pos_total, slot_total=slot_total,
                st_cols=st_cols, st_total=off, pt_max=PT_MAX)
    return shared, ins_per_core, meta


def _build_program(meta):
    import concourse.bacc as bacc
    import concourse.bass as bass
    import concourse.mybir as mybir
    from contextlib import ExitStack

    f16, f32, i16 = mybir.dt.float16, mybir.dt.float32, mybir.dt.int16
    AF = mybir.ActivationFunctionType
    OP = mybir.AluOpType
    AX = mybir.AxisListType

    passes = meta["passes"]
    pos_total = meta["pos_total"]
    slot_total = meta["slot_total"]
    st_cols = meta["st_cols"]
    ST_TOT = meta["st_total"]
    PT_MAX = meta["pt_max"]
    NCH = 128

    nc = bacc.Bacc("TRN2")
    tbl = nc.dram_tensor("tbl", [NCHUNK * (CHUNK + 1), NCH], f16, kind="ExternalInput")
    wT = nc.dram_tensor("wT", [NCH, NCH], f16, kind="ExternalInput")
    wl = nc.dram_tensor("wl", [NCH, NCH], f16, kind="ExternalInput")
    as_bd = nc.dram_tensor("as_bd", [NCH, HEADS], f16, kind="ExternalInput")
    ad_bd = nc.dram_tensor("ad_bd", [NCH, HEADS], f16, kind="ExternalInput")
    biasr = nc.dram_tensor("biasr", [128, NCH], f32, kind="ExternalInput")
    ident = nc.dram_tensor("ident", [128, 128], f16, kind="ExternalInput")
    stcat = nc.dram_tensor("stcat", [128, ST_TOT], f16, kind="ExternalInput")
    xoT_dr = [nc.dram_tensor(f"xoT{g}", [NCH, PT_MAX], f16, kind="ExternalInput")
              for g in range(NCHUNK)]
    idx_dr = [nc.dram_tensor(f"idx{g}", [128, slot_total[g] // 16], i16,
                             kind="ExternalInput") for g in range(NCHUNK)]
    midx_dr = [nc.dram_tensor(f"midx{g}", [128, P_OUT // 16], i16,
                              kind="ExternalInput") for g in range(NCHUNK)]
    out_dr = nc.dram_tensor("out", [P_OUT, NCH], f32, kind="ExternalOutput")
    import os as _os
    _pk = "ExternalOutput" if _os.environ.get("KDBG") else "Internal"
    ptab = [nc.dram_tensor(f"ptab{g}", [pos_total[g] + 128, TROW], f16,
                           kind=_pk) if _pk == "ExternalOutput" else
            nc.dram_tensor(f"ptab{g}", [pos_total[g] + 128, TROW], f16)
            for g in range(NCHUNK)]

    ctx = ExitStack()
    sb_t = lambda name, shape, dt: ctx.enter_context(nc.sbuf_tensor(name, shape, dt))
    ps_t = lambda name, shape, dt: ctx.enter_context(nc.psum_tensor(name, shape, dt))
    sem = lambda name: ctx.enter_context(nc.semaphore(name))

    with ctx:
        mx_sb = [sb_t(f"mx{i}", [128, 1, SB_SLOTS], f16) for i in range(2)]
        ix_sb = [sb_t(f"ix{i}", [128, SB_SLOTS // 16], i16) for i in range(2)]
        st_sb = sb_t("st_sb", [128, ST_TOT], f16)
        ws128 = sb_t("ws128", [128, 128], f16)
        wl_sb = sb_t("wl_sb", [128, 128], f16)
        wT_sb = sb_t("wT_sb", [128, 128], f16)
        id_sb = sb_t("id_sb", [128, 128], f16)
        ws4 = sb_t("ws4", [128, HEADS], f16)
        wd4 = sb_t("wd4", [128, HEADS], f16)
        asbd_sb = sb_t("asbd_sb", [128, HEADS], f16)
        adbd_sb = sb_t("adbd_sb", [128, HEADS], f16)
        biasr_sb = sb_t("biasr_sb", [128, NCH], f32)
        NCHK = PT_MAX // 128
        dbcT = sb_t("dbcT", [128, NCHK * 128], f16)
        xo_buf = sb_t("xo_buf", [128, PT_MAX], f16)
        d4c = sb_t("d4c", [HEADS, 2048], f16)
        bb_sb = [sb_t(f"bb{i}", [128, 128], f16) for i in range(2)]
        lr_sb = [sb_t(f"lr{i}", [128, 1024], f16) for i in range(2)]
        ex_sb = [sb_t(f"ex{i}", [128, 1024], f16) for i in range(2)]
        mw_sb = [sb_t(f"mw{i}", [128, 1024], f16) for i in range(2)]
        strip = sb_t("strip", [128, SB_SLOTS], f16)
        zstrip = sb_t("zstrip", [128, SB_SLOTS], f16)
        stg = [sb_t(f"stg{i}", [128, TROW], f16) for i in range(2)]
        mg_t = [sb_t(f"mgt{i}", [128, MQR // 128, TROW], f16) for i in range(NCHUNK)]
        macc = sb_t("macc", [128, MQR // 128, NCH], f32)
        mz = sb_t("mz", [128, MQR // 128, HEADS], f32)
        mrz = sb_t("mrz", [128, MQR // 128, HEADS], f32)
        midx_sb = [sb_t(f"midx_sb{i}", [128, P_OUT // 16], i16) for i in range(2)]

        absum = [ps_t(f"absum{i}", [128, 1024], f32) for i in range(2)]
        he_ps = [ps_t(f"he{i}", [128, 512], f32) for i in range(2)]
        tp_ps = [ps_t(f"tp{i}", [128, 512], f16) for i in range(2)]

        cns, zb, xo, mm, aa, vv, pio, ow = (sem(s) for s in
            ("cns", "zb", "xo", "mm", "aa", "vv", "pio", "ow"))
        gios = [sem("gio0"), sem("gio1")]
        stws = [sem("stw0"), sem("stw1")]
        bd = [sem("bd0"), sem("bd1")]
        ccns, czb, cxo, cmm, caa, cvv, cpio, cow = [0], [0], [0], [0], [0], [0], [0], [0]
        cgios = [0, 0]
        cstw = [0, 0]
        cbd = [0, 0]

        def dma(dst, src):
            nc.sync.dma_start(dst, src).then_inc(cns, 16)
            ccns[0] += 16

        # ---------------- Phase A: constants ----------------
        dma(wT_sb[:, :], wT[:, :])
        dma(wl_sb[:, :], wl[:, :])
        dma(id_sb[:, :], ident[:, :])
        dma(asbd_sb[:, :], as_bd[:, :])
        dma(adbd_sb[:, :], ad_bd[:, :])
        dma(st_sb[:, :], stcat[:, :])
        dma(biasr_sb[:, :], biasr[:, :])
        nc.tensor.wait_ge(cns, ccns[0])
        nc.tensor.matmul(absum[0][:, 0:HEADS], wT_sb[:, :], asbd_sb[:, :],
                         start=True, stop=True).then_inc(mm, 1); cmm[0] += 1
        nc.tensor.matmul(absum[0][:, 4:4 + HEADS], wT_sb[:, :], adbd_sb[:, :],
                         start=True, stop=True).then_inc(mm, 1); cmm[0] += 1
        nc.scalar.wait_ge(mm, cmm[0])
        nc.scalar.activation(ws4[:, :], absum[0][:, 0:HEADS], AF.Copy).then_inc(aa, 1)
        nc.scalar.activation(wd4[:, :], absum[0][:, 4:4 + HEADS], AF.Copy).then_inc(aa, 1)
        caa[0] += 2
        nc.scalar.activation(ws128[:, :],
                             bass.AP(absum[0], 0, [[1024, 128], [1, HEADS], [0, 32]]),
                             AF.Copy, scale=NEG_SLOPE).then_inc(aa, 1); caa[0] += 1
        # zero-block rows for each table; staging tails stay zero forever
        nc.vector.memset(stg[0][:, :], 0.0)
        nc.vector.memset(stg[1][:, :], 0.0)
        nc.vector.memset(strip[:, :], 0.0)
        nc.vector.engine_nop().then_inc(vv, 1); cvv[0] += 1
        nc.sync.wait_ge(vv, cvv[0])
        for g in range(NCHUNK):
            nc.sync.dma_start(
                bass.AP(ptab[g], pos_total[g] * TROW, [[TROW, 128], [1, TROW]]),
                bass.AP(strip, 0, [[SB_SLOTS, 128], [1, TROW]]),
            ).then_inc(zb, 16); czb[0] += 16

        # ---------------- main passes ----------------
        first_blk = True
        ai = hi = li = si = 0
        rel_absum = [0, 0]   # aa counts releasing each absum buf
        rel_he = [0, 0]      # vv counts releasing he bufs
        rel_ex = [0, 0]      # vv counts releasing ex bufs
        rel_mx = [0, 0]      # mm counts releasing mx bufs
        rel_stg = [0, 0]     # io counts releasing stg bufs
        rel_strip = 0        # mm count releasing strip/zstrip
        rel_ix = [0, 0]
        rel_bb = [0, 0]
        rel_tp = [0, 0]

        for g in range(NCHUNK):
            # -- rebuild dbcT for this pass (position order) --
            nc.sync.wait_ge(mm, cmm[0])  # prior pass PE use of dbcT done
            nc.sync.dma_start(xo_buf[:, :], xoT_dr[g][:, :]).then_inc(xo, 16)
            cxo[0] += 16
            NR = PT_MAX // 2048
            for r in range(NR):
                c0 = r * 2048
                nc.tensor.wait_ge(xo, cxo[0])
                nc.tensor.wait_ge(aa, caa[0])
                for qh in range(4):
                    nc.tensor.matmul(absum[qh // 2][0:HEADS, (qh % 2) * 512:(qh % 2) * 512 + 512],
                                     wd4[:, :], xo_buf[:, c0 + qh * 512:c0 + qh * 512 + 512],
                                     start=True, stop=True).then_inc(mm, 1); cmm[0] += 1
                nc.scalar.wait_ge(mm, cmm[0])
                nc.scalar.wait_ge(bd[0], cbd[0])
                nc.scalar.wait_ge(bd[1], cbd[1])
                nc.scalar.activation(d4c[:, 0:1024], absum[0][0:HEADS, 0:1024],
                                     AF.Copy).then_inc(aa, 1)
                nc.scalar.activation(d4c[:, 1024:2048], absum[1][0:HEADS, 0:1024],
                                     AF.Copy).then_inc(aa, 1)
                caa[0] += 2
                nc.sync.wait_ge(aa, caa[0])
                for ch in range(16):
                    B = bb_sb[ch % 2]
                    nc.sync.wait_ge(mm, rel_bb[ch % 2])
                    nc.sync.dma_start(
                        B[:, :],
                        bass.AP(d4c, ch * 128, [[2048, 4], [0, 32], [1, 128]]),
                    ).then_inc(bd[ch % 2], 16); cbd[ch % 2] += 16
                    nc.tensor.wait_ge(bd[ch % 2], cbd[ch % 2])
                    nc.tensor.wait_ge(aa, rel_tp[ch % 2])
                    nc.tensor.transpose(tp_ps[ch % 2][:, 0:128], B[:, :], id_sb[:, :]
                                        ).then_inc(mm, 1); cmm[0] += 1
                    rel_bb[ch % 2] = cmm[0]
                    nc.scalar.wait_ge(mm, cmm[0])
                    nc.scalar.activation(
                        dbcT[:, (r * 16 + ch) * 128:(r * 16 + ch) * 128 + 128],
                        tp_ps[ch % 2][:, 0:128], AF.Copy, scale=NEG_SLOPE
                        ).then_inc(aa, 1); caa[0] += 1
                    rel_tp[ch % 2] = caa[0]
            rel_absum = [caa[0], caa[0]]

            # -- superblocks --
            for sb in passes[g]:
                D, npb, colsb = sb["D"], sb["npb"], sb["colsb"]
                st0, stw = st_cols[D]
                nblk = sb["nodes"] // npb
                sp = sb["slots_pad"]
                b_ix = si % 2
                b_mx = si % 2
                nc.gpsimd.wait_ge(gios[b_ix], cgios[b_ix])
                nc.gpsimd.dma_start(ix_sb[b_ix][:, 0:sp // 16],
                                    idx_dr[g][:, sb["slot0"] // 16:(sb["slot0"] + sp) // 16]
                                    ).then_inc(pio, 16); cpio[0] += 16
                nc.gpsimd.wait_ge(pio, cpio[0])
                nc.gpsimd.wait_ge(mm, rel_mx[b_mx])
                nc.gpsimd.dma_gather(
                    mx_sb[b_mx][:, :, 0:sp],
                    tbl[g * (CHUNK + 1):(g + 1) * (CHUNK + 1), :],
                    ix_sb[b_ix][:, 0:sp // 16],
                    sp, sp, NCH, transpose=True, single_packet=False,
                ).then_inc(gios[b_mx], 16); cgios[b_mx] += 16
                gwait = (b_mx, cgios[b_mx])
                si += 1

                # blocks
                nc.vector.wait_ge(mm, rel_strip)  # strips free (prev sb transposed)
                for b in range(nblk):
                    k0 = b * colsb
                    q = (b * npb) % 128
                    cchunk = ((sb["pos0"] + b * npb) // 128)
                    qq = (sb["pos0"] + b * npb) % 128
                    A = absum[ai % 2]
                    nc.tensor.wait_ge(gios[gwait[0]], gwait[1])
                    nc.tensor.wait_ge(aa, rel_absum[ai % 2])
                    L = li % 2
                    chunks = list(range(0, colsb, 512))
                    mm_d = {}
                    # 1) alpha s+d accumulation, all chunks
                    for c0a in chunks:
                        cwa = min(512, colsb - c0a)
                        nc.tensor.matmul(A[:, c0a:c0a + cwa], ws128[:, :],
                                         mx_sb[b_mx][:, 0, k0 + c0a:k0 + c0a + cwa],
                                         start=True, stop=False).then_inc(mm, 1); cmm[0] += 1
                        nc.tensor.matmul(A[:, c0a:c0a + cwa],
                                         dbcT[qq:qq + npb, cchunk * 128:cchunk * 128 + 128],
                                         st_sb[qq:qq + npb, st0 + c0a:st0 + c0a + cwa],
                                         start=False, stop=True,
                                         tile_position=(qq, 0),
                                         ).then_inc(mm, 1); cmm[0] += 1
                        mm_d[c0a] = cmm[0]
                    # 2) he-mms fill PE while ACT runs relu below
                    he_of = {}
                    for c0h in chunks:
                        cwh = min(512, colsb - c0h)
                        H = hi % 2
                        nc.tensor.wait_ge(vv, rel_he[H])
                        nc.tensor.matmul(he_ps[H][:, 0:cwh], wl_sb[:, :],
                                         mx_sb[b_mx][:, 0, k0 + c0h:k0 + c0h + cwh],
                                         start=True, stop=True).then_inc(mm, 1); cmm[0] += 1
                        he_of[c0h] = (H, cmm[0])
                        hi += 1
                    # 3) relu (ACT, overlapped with he) then L-mm then exp
                    aa_r = {}
                    nc.scalar.wait_ge(vv, rel_ex[L])
                    for c0a in chunks:
                        cwa = min(512, colsb - c0a)
                        nc.scalar.wait_ge(mm, mm_d[c0a])
                        nc.scalar.activation(lr_sb[L][:, c0a:c0a + cwa],
                                             A[:, c0a:c0a + cwa],
                                             AF.Relu, scale=4.0).then_inc(aa, 1)
                        caa[0] += 1; aa_r[c0a] = caa[0]
                    mm_l = {}
                    for c0a in chunks:
                        cwa = min(512, colsb - c0a)
                        nc.tensor.wait_ge(aa, aa_r[c0a])
                        nc.tensor.matmul(A[:, c0a:c0a + cwa], id_sb[:, :],
                                         lr_sb[L][:, c0a:c0a + cwa],
                                         start=False, stop=True,
                                         skip_group_check=True).then_inc(mm, 1); cmm[0] += 1
                        mm_l[c0a] = cmm[0]
                    for c0a in chunks:
                        cwa = min(512, colsb - c0a)
                        nc.scalar.wait_ge(mm, mm_l[c0a])
                        nc.scalar.activation(ex_sb[L][:, c0a:c0a + cwa],
                                             A[:, c0a:c0a + cwa],
                                             AF.Exp).then_inc(aa, 1); caa[0] += 1
                    rel_absum[ai % 2] = caa[0]
                    ai += 1
                    # 4) weighted messages
                    for c0h in chunks:
                        cwh = min(512, colsb - c0h)
                        H, mmh = he_of[c0h]
                        nc.vector.wait_ge(mm, mmh)
                        nc.vector.wait_ge(aa, caa[0])
                        nc.vector.tensor_tensor(mw_sb[L][:, c0h:c0h + cwh],
                                                he_ps[H][:, 0:cwh],
                                                ex_sb[L][:, c0h:c0h + cwh], OP.mult
                                                ).then_inc(vv, 1); cvv[0] += 1
                        rel_he[H] = cvv[0]
                    nc.vector.wait_ge(vv, cvv[0])
                    with nc.allow_low_precision(reason="fp16 table rows"):
                        nc.vector.tensor_reduce(
                            strip[:, b * npb:b * npb + npb],
                            mw_sb[L][:, 0:colsb].rearrange("p (n d) -> p n d", d=D),
                            AX.X, OP.add).then_inc(vv, 1); cvv[0] += 1
                        nc.vector.tensor_reduce(
                            zstrip[:, b * npb:b * npb + npb],
                            ex_sb[L][:, 0:colsb].rearrange("p (n d) -> p n d", d=D),
                            AX.X, OP.add).then_inc(vv, 1); cvv[0] += 1
                    rel_ex[L] = cvv[0]
                    li += 1
                rel_mx[b_mx] = cmm[0]

                # transpose strips -> staging -> table rows
                ntile = -(-sb["nodes"] // 128)
                vwait = cvv[0]
                for t in range(ntile):
                    tw = min(128, sb["nodes"] - t * 128)
                    T = tp_ps[t % 2]
                    S = stg[t % 2]
                    nc.tensor.wait_ge(vv, vwait)
                    nc.tensor.wait_ge(aa, rel_tp[t % 2])
                    nc.tensor.transpose(T[0:tw, 0:128],
                                        strip[:, t * 128:t * 128 + tw], id_sb[:, :]
                                        ).then_inc(mm, 1); cmm[0] += 1
                    TZ = T
                    nc.tensor.transpose(TZ[0:tw, 128:256],
                                        zstrip[:, t * 128:t * 128 + tw], id_sb[:, :]
                                        ).then_inc(mm, 1); cmm[0] += 1
                    nc.scalar.wait_ge(mm, cmm[0])
                    nc.scalar.wait_ge(stws[t % 2], rel_stg[t % 2])
                    nc.scalar.activation(S[0:tw, 0:128], T[0:tw, 0:128], AF.Copy).then_inc(aa, 1)
                    nc.scalar.activation(S[0:tw, 128:132],
                                         bass.AP(TZ, 128, [[512, tw], [32, 4]]),
                                         AF.Copy).then_inc(aa, 1)
                    caa[0] += 2
                    rel_tp[t % 2] = caa[0]
                    nc.sync.wait_ge(aa, caa[0])
                    nc.sync.dma_start(
                        bass.AP(ptab[g], (sb["pos0"] + t * 128) * TROW,
                                [[TROW, tw], [1, TROW]]),
                        S[0:tw, 0:TROW],
                    ).then_inc(stws[t % 2], 16); cstw[t % 2] += 16
                    rel_stg[t % 2] = cstw[t % 2]
                rel_strip = cmm[0]

        # ---------------- merge ----------------
        nc.gpsimd.wait_ge(stws[0], cstw[0])
        nc.gpsimd.wait_ge(stws[1], cstw[1])
        nc.gpsimd.wait_ge(zb, czb[0])
        nc.gpsimd.wait_ge(vv, cvv[0])
        rel_mg = 0
        for rnd in range(P_OUT // MQR):
            r0 = rnd * MQR
            nc.gpsimd.wait_ge(vv, rel_mg)
            for g in range(NCHUNK):
                nc.gpsimd.wait_ge(gios[g % 2], cgios[g % 2])
                nc.gpsimd.dma_start(
                    midx_sb[g % 2][:, 0:MQR // 16],
                    midx_dr[g][:, r0 // 16:(r0 + MQR) // 16]).then_inc(pio, 16)
                cpio[0] += 16
                nc.gpsimd.wait_ge(pio, cpio[0])
                nc.gpsimd.dma_gather(
                    mg_t[g][:, :, :], ptab[g][:, :], midx_sb[g % 2][:, 0:MQR // 16],
                    MQR, MQR, TROW, transpose=False, single_packet=False,
                ).then_inc(gios[g % 2], 16); cgios[g % 2] += 16
            nc.vector.wait_ge(gios[0], cgios[0])
            nc.vector.wait_ge(gios[1], cgios[1])
            nc.vector.wait_ge(ow, cow[0])
            vself = cvv[0]
            nc.vector.tensor_tensor(macc[:, :, :], mg_t[0][:, :, 0:NCH],
                                    mg_t[1][:, :, 0:NCH], OP.add).then_inc(vv, 1)
            nc.vector.wait_ge(vv, cvv[0] + 1)
            nc.vector.tensor_tensor(macc[:, :, :], macc[:, :, :],
                                    mg_t[2][:, :, 0:NCH], OP.add).then_inc(vv, 1)
            nc.vector.wait_ge(vv, cvv[0] + 2)
            nc.vector.tensor_tensor(macc[:, :, :], macc[:, :, :],
                                    mg_t[3][:, :, 0:NCH], OP.add).then_inc(vv, 1)
            nc.vector.tensor_tensor(mz[:, :, :], mg_t[0][:, :, NCH:NCH + HEADS],
                                    mg_t[1][:, :, NCH:NCH + HEADS], OP.add).then_inc(vv, 1)
            nc.vector.wait_ge(vv, cvv[0] + 4)
            nc.vector.tensor_tensor(mz[:, :, :], mz[:, :, :],
                                    mg_t[2][:, :, NCH:NCH + HEADS], OP.add).then_inc(vv, 1)
            nc.vector.wait_ge(vv, cvv[0] + 5)
            nc.vector.tensor_tensor(mz[:, :, :], mz[:, :, :],
                                    mg_t[3][:, :, NCH:NCH + HEADS], OP.add).then_inc(vv, 1)
            nc.vector.wait_ge(vv, cvv[0] + 6)
            nc.vector.tensor_scalar_add(mz[:, :, :], mz[:, :, :], 1e-20).then_inc(vv, 1)
            cvv[0] += 7
            rel_mg = cvv[0]
            nc.vector.wait_ge(vv, cvv[0])
            nc.vector.reciprocal(mrz[:, :, :], mz[:, :, :]).then_inc(vv, 1); cvv[0] += 1
            nc.vector.wait_ge(vv, cvv[0])
            nc.vector.tensor_tensor(
                macc[:, :, :], macc[:, :, :],
                bass.AP(mrz, 0, [[(MQR // 128) * HEADS, 128], [HEADS, MQR // 128],
                                 [1, HEADS], [0, 32]]),
                OP.mult).then_inc(vv, 1); cvv[0] += 1
            nc.vector.wait_ge(vv, cvv[0])
            nc.vector.tensor_tensor(
                macc[:, :, :], macc[:, :, :],
                bass.AP(biasr_sb, 0, [[NCH, 128], [0, MQR // 128], [1, NCH]]),
                OP.add).then_inc(vv, 1); cvv[0] += 1
            nc.sync.wait_ge(vv, cvv[0])
            nc.sync.dma_start(
                bass.AP(out_dr, r0 * NCH,
                        [[NCH, 128], [128 * NCH, MQR // 128], [1, NCH]]),
                macc[:, :, :],
            ).then_inc(ow, 16); cow[0] += 16
        nc.sync.wait_ge(ow, cow[0])
        nc.gpsimd.wait_ge(ow, cow[0])

    nc.compile()
    return nc


_CACHE = {}


def kernel(x, edge_index, weight, att, bias):
    import sys
    if '/opt/trn_rl_repo' not in sys.path:
        sys.path.insert(0, '/opt/trn_rl_repo')
    from concourse.bass_utils import run_bass_kernel_spmd

    shared, per_core, meta = _host_prep(x, edge_index, weight, att, bias)
    key = "prog"
    if key not in _CACHE:
        _CACHE[key] = _build_program(meta)
    nc = _CACHE[key]
    in_maps = [dict(shared, **per_core[c]) for c in range(NCORES)]
    res = run_bass_kernel_spmd(nc, in_maps, list(range(NCORES)))
    outs = [res.results[c]["out"][:NPC, :] for c in range(NCORES)]
    return np.concatenate(outs, axis=0).astype(np.float32)



# revision 5
# speedup vs baseline: 3.0660x; 3.0660x over previous
"""GATConv Trainium2 kernel — 8-core SPMD, dst-sharded, scatter-add design.

Sharding: dst nodes partitioned across 8 cores (12.5K each); every core
owns all edges into its nodes (no collectives); the fp16 node-feature
table is replicated per core for src gathers (4 chunks of 32767 rows so
gather indices fit int16).

Per core, per src-chunk pass: edges grouped by exact per-pass dst degree
D into uniform 128-slot tiles (npb = 128//D nodes per tile, run-major).
Per tile the PE computes, with the gathered x_src^T chunk as the
stationary operand, he^T = x^T W (128 cols) and alpha^T = x^T (W a_s)
(4 cols); a second accumulating matmul with an identity stationary adds
the per-dst attention term (gathered per-slot from a small on-device
d4 table). ACT applies Prelu(0.2) + Exp on the 4-col transposed logits
(segment-max is skipped: logits are O(1) and softmax is shift
invariant), DVE forms mw = he*ex, and a per-class column-shifted wide
staircase matmul segment-sums [agg|z] rows node-major into PSUM fills
of 128 packed nodes. Fills are copied to fp16 staging and DMA
scatter-added into a per-pass sub-row of a DRAM accumulator table
(indices within a call are unique so the adds cannot race). A final
sequential phase sums the 4 sub-rows, normalizes by z and adds bias.
"""

import numpy as np

N = 100000
E = 1600000
IN_CH = 128
HEADS = 4
OUT_CH = 32
NEG_SLOPE = 0.2
NCORES = 8
NPC = N // NCORES            # 12500 nodes per core
CHUNK = 32767                # real rows per gather-table chunk
NCHUNK = 4
NLOC_PAD = 12544             # 98 * 128, x_own padded cols
TBL_ROWS = 12928             # 101 * 128 accumulator rows
DUMP = 12800                 # scatter dump row for padding nodes
ROWW = 640                   # fp16 elems per accumulator row (1280 B)
SUBW = 160                   # fp16 elems per pass sub-row (132 used)
ZROW = 12500                 # zero row of the d4 table
GST = 32                     # tiles per gather superblock
SGF = 16                     # fills per scatter group
FINR = 10                    # final-phase rounds (1280 nodes each)


def _wrap16(a):
    S = len(a)
    w = np.empty((128, S // 16), np.int16)
    t = a.astype(np.int16).reshape(S // 16, 16).T
    for r in range(8):
        w[16 * r:16 * r + 16] = t
    return np.ascontiguousarray(w)


def _host_prep(x, edge_index, weight, att, bias):
    x = np.asarray(x, np.float32)
    ei = np.asarray(edge_index)
    src = ei[0].astype(np.int64)
    dst = ei[1].astype(np.int64)
    weight = np.asarray(weight, np.float32)
    att = np.asarray(att, np.float32)
    bias = np.asarray(bias, np.float32)

    tbl = np.zeros((NCHUNK * (CHUNK + 1), IN_CH), np.float16)
    for g in range(NCHUNK):
        lo, hi = g * CHUNK, min((g + 1) * CHUNK, N)
        if lo < N:
            tbl[g * (CHUNK + 1): g * (CHUNK + 1) + (hi - lo)] = x[lo:hi].astype(np.float16)
    src_chunk = (src // CHUNK).astype(np.int32)
    src_local = (src % CHUNK).astype(np.int32)
    core = (dst // NPC).astype(np.int32)
    dstl = (dst % NPC).astype(np.int32)

    flat = (core.astype(np.int64) * NCHUNK + src_chunk) * NPC + dstl
    counts = np.bincount(flat, minlength=NCORES * NCHUNK * NPC).reshape(
        NCORES, NCHUNK, NPC).astype(np.int32)
    maxd = int(counts.max())
    assert maxd <= 127, f"per-pass degree {maxd} > 127"

    nclass = np.zeros((NCORES, NCHUNK, maxd + 1), np.int64)
    for c in range(NCORES):
        for g in range(NCHUNK):
            nclass[c, g] = np.bincount(counts[c, g], minlength=maxd + 1)
    uni = nclass.max(axis=0)  # [NCHUNK, maxd+1]

    used_D = sorted({D for g in range(NCHUNK) for D in range(1, maxd + 1)
                     if uni[g, D] > 0})
    st2_col0 = {}
    st_list = []
    off = 0
    for D in used_D:
        npb = 128 // D
        w = np.zeros((128, 256), np.float16)
        s = np.arange(D * npb)
        w[s, 128 + s // D] = 1.0
        st_list.append(w)
        st2_col0[D] = off
        off += 256
    st2cat = np.concatenate(st_list, axis=1)

    def _gran(npb):
        if npb > 64:
            return 128
        if npb > 32:
            return 64
        return 32

    # canonical tile/fill layout per pass (uniform across cores); tiles
    # occupy power-of-2 partition granules so stationary windows stay
    # 32-aligned for the PE
    passes = []
    sd_keys = set()
    for g in range(NCHUNK):
        tiles = []
        for D in used_D:
            nn = int(uni[g, D])
            if nn == 0:
                continue
            npb = 128 // D
            for i in range(-(-nn // npb)):
                tiles.append(dict(D=D, npb=npb, kbase=i * npb))
        o = 0
        fill = 0
        for t in tiles:
            gr = _gran(t["npb"])
            o = -(-o // gr) * gr
            if o + gr > 128:
                fill += 1
                o = 0
            t["o"] = o
            t["fill"] = fill
            sd_keys.add((t["D"], o))
            o += gr
        nfills = fill + 1
        for i, t in enumerate(tiles):
            t["fill_start"] = (i == 0) or (tiles[i - 1]["fill"] != t["fill"])
            t["fill_end"] = (i == len(tiles) - 1) or (tiles[i + 1]["fill"] != t["fill"])
        nsg = -(-nfills // SGF)
        passes.append(dict(tiles=tiles, ntiles=len(tiles), nfills=nfills, nsg=nsg))

    ins_per_core = []
    for c in range(NCORES):
        ci = {}
        xo = np.zeros((NLOC_PAD, IN_CH), np.float16)
        xo[:NPC] = x[c * NPC:(c + 1) * NPC].astype(np.float16)
        ci["xoT"] = np.ascontiguousarray(xo.T)
        for g in range(NCHUNK):
            pm = passes[g]
            m = (core == c) & (src_chunk == g)
            e_dst = dstl[m]
            e_srcl = src_local[m]
            o = np.argsort(e_dst, kind="stable")
            e_dst, e_srcl = e_dst[o], e_srcl[o]
            starts = np.searchsorted(e_dst, np.arange(NPC + 1))
            cnts = counts[c, g]
            byD = {D: np.nonzero(cnts == D)[0] for D in used_D}

            gx = np.full(128 * pm["ntiles"], CHUNK, np.int32)
            gp = np.full(128 * pm["nsg"] * SGF, ZROW, np.int32)
            sc = np.full(128 * pm["nsg"] * SGF, DUMP, np.int32)
            for ti, t in enumerate(pm["tiles"]):
                D, npb, kbase = t["D"], t["npb"], t["kbase"]
                nodes = byD[D][kbase:kbase + npb]
                base = ti * 128
                for k, node in enumerate(nodes):
                    s0 = starts[node]
                    gx[base + k * D: base + (k + 1) * D] = e_srcl[s0:s0 + D]
                    gp[t["fill"] * 128 + t["o"] + k] = node
                    sc[t["fill"] * 128 + t["o"] + k] = node
            ci[f"gx{g}"] = _wrap16(gx)
            ci[f"gp{g}"] = _wrap16(gp)
            ci[f"sc{g}"] = _wrap16(sc)
        ins_per_core.append(ci)

    att_src = att[0, :, :OUT_CH]
    att_dst = att[0, :, OUT_CH:]
    as_bd = np.zeros((IN_CH, HEADS), np.float16)
    ad_bd = np.zeros((IN_CH, HEADS), np.float16)
    for h in range(HEADS):
        as_bd[32 * h:32 * h + 32, h] = att_src[h].astype(np.float16)
        ad_bd[32 * h:32 * h + 32, h] = att_dst[h].astype(np.float16)

    sd_list = sorted(sd_keys)
    sd_col0 = {}
    sds = []
    for vi, (D, o) in enumerate(sd_list):
        npb = 128 // D
        w = np.zeros((128, 128), np.float16)
        s = np.arange(D * npb)
        w[o + s // D, s] = 1.0
        sds.append(w)
        sd_col0[(D, o)] = vi * 128
    sdcat = np.concatenate(sds, axis=1)

    shared = {
        "tbl": tbl,
        "sdcat": sdcat,
        "wT": np.ascontiguousarray(weight.T).astype(np.float16),
        "wl": weight.astype(np.float16),
        "as_bd": as_bd, "ad_bd": ad_bd,
        "biasr": np.tile(bias.astype(np.float32)[None, :], (128, 1)),
        "ident": np.eye(128, dtype=np.float16),
        "st2cat": st2cat,
    }
    meta = dict(passes=passes, st2_col0=st2_col0, st2_w=st2cat.shape[1],
                sd_col0=sd_col0, sd_w=sdcat.shape[1])
    return shared, ins_per_core, meta


def _build_program(meta):
    import concourse.bacc as bacc
    import concourse.bass as bass
    import concourse.mybir as mybir
    from contextlib import ExitStack

    f16, f32, i16 = mybir.dt.float16, mybir.dt.float32, mybir.dt.int16
    AF = mybir.ActivationFunctionType
    OP = mybir.AluOpType

    passes = meta["passes"]
    st2_col0 = meta["st2_col0"]
    ST2W = meta["st2_w"]
    sd_col0 = meta["sd_col0"]
    SDW = meta["sd_w"]
    FCH = 32
    NCH = 128
    MAXT = max(pm["ntiles"] for pm in passes)
    MAXSG = max(pm["nsg"] for pm in passes)

    nc = bacc.Bacc("TRN2")
    tbl = nc.dram_tensor("tbl", [NCHUNK * (CHUNK + 1), NCH], f16, kind="ExternalInput")
    wT = nc.dram_tensor("wT", [NCH, NCH], f16, kind="ExternalInput")
    wl = nc.dram_tensor("wl", [NCH, NCH], f16, kind="ExternalInput")
    as_bd = nc.dram_tensor("as_bd", [NCH, HEADS], f16, kind="ExternalInput")
    ad_bd = nc.dram_tensor("ad_bd", [NCH, HEADS], f16, kind="ExternalInput")
    biasr = nc.dram_tensor("biasr", [128, NCH], f32, kind="ExternalInput")
    ident = nc.dram_tensor("ident", [128, 128], f16, kind="ExternalInput")
    st2cat = nc.dram_tensor("st2cat", [128, ST2W], f16, kind="ExternalInput")
    xoT = nc.dram_tensor("xoT", [NCH, NLOC_PAD], f16, kind="ExternalInput")
    gx_dr = [nc.dram_tensor(f"gx{g}", [128, passes[g]["ntiles"] * 8], i16,
                            kind="ExternalInput") for g in range(NCHUNK)]
    gp_dr = [nc.dram_tensor(f"gp{g}", [128, passes[g]["nsg"] * SGF * 8], i16,
                            kind="ExternalInput") for g in range(NCHUNK)]
    sdcat = nc.dram_tensor("sdcat", [128, SDW], f16, kind="ExternalInput")
    sc_dr = [nc.dram_tensor(f"sc{g}", [128, passes[g]["nsg"] * SGF * 8], i16,
                            kind="ExternalInput") for g in range(NCHUNK)]
    out_dr = nc.dram_tensor("out", [FINR * 1280, NCH], f32, kind="ExternalOutput")
    import os as _os
    _dbg = bool(_os.environ.get("KDBG"))
    acc_dr = nc.dram_tensor("acc", [TBL_ROWS, ROWW], f16,
                            kind="ExternalOutput" if _dbg else "Internal")
    d4_dr = nc.dram_tensor("d4t", [NLOC_PAD, 128], f16,
                           kind="ExternalOutput" if _dbg else "Internal")

    ctx = ExitStack()
    sb_t = lambda name, shape, dt: ctx.enter_context(nc.sbuf_tensor(name, shape, dt))
    ps_t = lambda name, shape, dt: ctx.enter_context(nc.psum_tensor(name, shape, dt))
    sem = lambda name: ctx.enter_context(nc.semaphore(name))

    with ctx:
        wl_sb = sb_t("wl_sb", [128, 128], f16)
        wT_sb = sb_t("wT_sb", [128, 128], f16)
        id_sb = sb_t("id_sb", [128, 128], f16)
        asbd_sb = sb_t("asbd_sb", [128, HEADS], f16)
        adbd_sb = sb_t("adbd_sb", [128, HEADS], f16)
        biasr_sb = sb_t("biasr_sb", [128, NCH], f32)
        st2_sb = sb_t("st2_sb", [128, ST2W], f16)
        ws4_sb = sb_t("ws4_sb", [128, HEADS], f16)
        wd4_sb = sb_t("wd4_sb", [128, HEADS], f16)
        xo_sb = sb_t("xo_sb", [128, NLOC_PAD], f16)
        d4n_sb = sb_t("d4n_sb", [128, (NLOC_PAD // 128) * HEADS], f16)
        zero_sb = sb_t("zero_sb", [128, ROWW], f16)
        mx_sb = [sb_t(f"mx{i}", [128, 1, GST * 128], f16) for i in range(2)]
        gxi_sb = [sb_t(f"gxi{i}", [128, GST * 8], i16) for i in range(2)]
        gpi_sb = [sb_t(f"gpi{i}", [128, MAXSG * SGF * 8], i16) for i in range(2)]
        sd_sb = sb_t("sd_sb", [128, SDW], f16)
        sci_sb = [sb_t(f"sci{i}", [128, MAXSG * SGF * 8], i16) for i in range(2)]
        d4g_sb = [sb_t(f"d4g{i}", [128, FCH, 128], f16) for i in range(2)]
        lr_sb = [sb_t(f"lr{i}", [128, 128], f16) for i in range(2)]
        mw_sb = sb_t("mw_sb", [128, 8 * 132], f16)
        stg_sb = [sb_t(f"stg{i}", [128, SGF, 132], f16) for i in range(2)]
        acc_sb = [sb_t(f"accsb{i}", [128, FINR, ROWW], f16) for i in range(2)]
        t01_sb = sb_t("t01_sb", [128, FINR, 132], f16)
        t23_sb = sb_t("t23_sb", [128, FINR, 132], f16)
        ts_sb = sb_t("ts_sb", [128, FINR, 132], f16)
        zf_sb = sb_t("zf_sb", [128, FINR, HEADS], f32)
        rz_sb = sb_t("rz_sb", [128, FINR, HEADS], f32)
        of_sb = [sb_t(f"of{i}", [128, FINR, NCH], f32) for i in range(2)]

        he_ps = ps_t("he_ps", [128, 1024], f32)
        al_ps = ps_t("al_ps", [128, 256], f32)
        agg_ps = ps_t("agg_ps", [128, 512], f32)
        ph_ps = ps_t("ph_ps", [128, 136], f32)

        cns, mm, aa, vv, gd_s, sc_s, ow = (sem(s) for s in
            ("cns", "mm", "aa", "vv", "gd_s", "sc_s", "ow"))
        gx_s = [sem("gx0"), sem("gx1")]
        C = dict(cns=0, mm=0, aa=0, vv=0, gd_s=0, sc_s=0, ow=0)
        cgx = [0, 0]

        def dma(dst, src):
            nc.sync.dma_start(dst, src).then_inc(cns, 16)
            C["cns"] += 16

        # ---------------- Phase A ----------------
        dma(wl_sb[:, :], wl[:, :])
        dma(wT_sb[:, :], wT[:, :])
        dma(id_sb[:, :], ident[:, :])
        dma(asbd_sb[:, :], as_bd[:, :])
        dma(adbd_sb[:, :], ad_bd[:, :])
        dma(biasr_sb[:, :], biasr[:, :])
        dma(st2_sb[:, :], st2cat[:, :])
        dma(sd_sb[:, :], sdcat[:, :])
        dma(xo_sb[:, :], xoT[:, :])
        consts_done = C["cns"]
        nc.vector.memset(zero_sb[:, :], 0.0)
        nc.vector.engine_nop().then_inc(vv, 1); C["vv"] += 1
        nc.sync.wait_ge(vv, C["vv"])
        nc.sync.dma_start(
            bass.AP(acc_dr, 0, [[ROWW, 128], [ROWW * 128, TBL_ROWS // 128], [1, ROWW]]),
            bass.AP(zero_sb, 0, [[ROWW, 128], [0, TBL_ROWS // 128], [1, ROWW]]),
        ).then_inc(cns, 16); C["cns"] += 16
        zero_done = C["cns"]

        nc.tensor.wait_ge(cns, consts_done)
        nc.tensor.matmul(ph_ps[:, 128:132], wT_sb[:, :], asbd_sb[:, :],
                         start=True, stop=True).then_inc(mm, 1); C["mm"] += 1
        nc.tensor.matmul(ph_ps[:, 132:136], wT_sb[:, :], adbd_sb[:, :],
                         start=True, stop=True).then_inc(mm, 1); C["mm"] += 1
        nc.scalar.wait_ge(mm, C["mm"])
        nc.scalar.activation(ws4_sb[:, :], ph_ps[:, 128:132], AF.Copy).then_inc(aa, 1)
        nc.scalar.activation(wd4_sb[:, :], ph_ps[:, 132:136], AF.Copy).then_inc(aa, 1)
        C["aa"] += 2
        # d4[n, h] = sum_in xo[in, n] * wd4[in, h], 32 node-chunks per bank fill
        NCHK = NLOC_PAD // 128
        for b in range(-(-NCHK // 32)):
            lo, hi = b * 32, min(NCHK, b * 32 + 32)
            nc.tensor.wait_ge(aa, C["aa"])
            for ch in range(lo, hi):
                nc.tensor.matmul(ph_ps[:, 4 * (ch - lo):4 * (ch - lo) + 4],
                                 xo_sb[:, 128 * ch:128 * ch + 128], wd4_sb[:, :],
                                 start=True, stop=True).then_inc(mm, 1); C["mm"] += 1
            nc.scalar.wait_ge(mm, C["mm"])
            nc.scalar.activation(d4n_sb[:, 4 * lo:4 * hi],
                                 ph_ps[:, 0:4 * (hi - lo)], AF.Copy).then_inc(aa, 1)
            C["aa"] += 1
        nc.sync.wait_ge(aa, C["aa"])
        nc.sync.dma_start(
            bass.AP(d4_dr, 0, [[128, 128], [128 * 128, NCHK], [1, HEADS]]),
            bass.AP(d4n_sb, 0, [[NCHK * HEADS, 128], [HEADS, NCHK], [1, HEADS]]),
        ).then_inc(cns, 16); C["cns"] += 16

        # ---------------- Phase B: passes ----------------
        he_last = [0] * 8        # vv counts freeing he/mw slot k (mult done)
        mw_last = [0] * 8        # mm counts freeing mw slot k (stair done)
        al_free = [0, 0]         # aa counts freeing al_ps halves
        mx_free = [0, 0]         # mm counts freeing mx bufs
        d4g_free = [0, 0]        # mm counts freeing d4g chunk bufs
        gpi_free = [0, 0]        # gd_s counts freeing gpi bufs
        sci_free = [0, 0]        # sc_s counts freeing sci bufs
        stg_free = [0, 0]        # sc_s counts freeing stg bufs
        fc_done = {}             # global fill -> aa count of staging copy
        gfill = 0

        for g in range(NCHUNK):
            pm = passes[g]
            tiles, ntiles, nfills, nsg = (pm[k] for k in
                                          ("tiles", "ntiles", "nfills", "nsg"))
            par = g % 2
            gf0 = gfill
            nc.sync.wait_ge(gd_s, gpi_free[par])
            dma(gpi_sb[par][:, 0:nsg * SGF * 8], gp_dr[g][:, :])
            nc.sync.wait_ge(sc_s, sci_free[par])
            dma(sci_sb[par][:, 0:nsg * SGF * 8], sc_dr[g][:, :])
            gpi_done = C["cns"]
            if g >= 1:
                # serialize cross-pass scatters (sub-rows share 256B blocks)
                nc.gpsimd.wait_ge(sc_s, C["sc_s"])

            ngs = -(-ntiles // GST)
            nch_d4 = -(-nfills // FCH)
            d4_ready = [0] * nch_d4

            def emit_d4(ck):
                if ck >= nch_d4:
                    return
                f0, f1 = ck * FCH, min(nfills, (ck + 1) * FCH)
                nf = f1 - f0
                p = ck % 2
                nc.gpsimd.wait_ge(cns, gpi_done)
                nc.gpsimd.wait_ge(mm, d4g_free[p])
                nc.gpsimd.dma_gather(
                    d4g_sb[p][:, 0:nf, :], d4_dr[:, :],
                    gpi_sb[par][:, f0 * 8:f1 * 8], 128 * nf, 128 * nf, 128,
                    transpose=False, single_packet=False,
                ).then_inc(gd_s, 16); C["gd_s"] += 16
                d4_ready[ck] = C["gd_s"]

            emit_d4(0)
            emit_d4(1)
            gpi_free[par] = C["gd_s"]

            def emit_gather(k):
                if k >= ngs:
                    return
                t0, t1 = k * GST, min(ntiles, (k + 1) * GST)
                sl = t1 - t0
                p = k % 2
                nc.sync.wait_ge(gx_s[p], cgx[p])
                dma(gxi_sb[p][:, 0:sl * 8], gx_dr[g][:, t0 * 8:t1 * 8])
                nc.gpsimd.wait_ge(cns, C["cns"])
                nc.gpsimd.wait_ge(mm, mx_free[p])
                nc.gpsimd.dma_gather(
                    mx_sb[p][:, :, 0:sl * 128],
                    tbl[g * (CHUNK + 1):(g + 1) * (CHUNK + 1), :],
                    gxi_sb[p][:, 0:sl * 8], sl * 128, sl * 128, NCH,
                    transpose=True, single_packet=False,
                ).then_inc(gx_s[p], 16); cgx[p] += 16

            emit_gather(0)
            emit_gather(1)

            cur_ck = -1
            for t_i, t in enumerate(tiles):
                gs = t_i // GST
                lt = t_i - gs * GST
                gsp = gs % 2
                k8 = t_i % 8
                f = t["fill"]
                ck = f // FCH
                if ck > cur_ck:
                    if cur_ck >= 0:
                        d4g_free[cur_ck % 2] = C["mm"]
                    cur_ck = ck
                    emit_d4(ck + 1)
                    nc.tensor.wait_ge(gd_s, d4_ready[ck])

                # he matmul (slot k8)
                if lt == 0:
                    nc.tensor.wait_ge(gx_s[gsp], cgx[gsp])
                nc.tensor.wait_ge(vv, he_last[k8])
                nc.tensor.matmul(he_ps[:, 128 * k8:128 * k8 + 128],
                                 mx_sb[gsp][:, 0, 128 * lt:128 * lt + 128],
                                 wl_sb[:, :], start=True, stop=True
                                 ).then_inc(mm, 1); C["mm"] += 1
                # alpha matmul + per-dst attention term via staircase stationary
                if lt == 0:
                    nc.tensor.wait_ge(aa, al_free[gsp])
                nc.tensor.matmul(al_ps[:, 128 * gsp + 4 * lt:128 * gsp + 4 * lt + 4],
                                 mx_sb[gsp][:, 0, 128 * lt:128 * lt + 128],
                                 ws4_sb[:, :], start=True, stop=False
                                 ).then_inc(mm, 1); C["mm"] += 1
                D, o, npb = t["D"], t["o"], t["npb"]
                c0d = sd_col0[(D, o)]
                nc.tensor.matmul(al_ps[:, 128 * gsp + 4 * lt:128 * gsp + 4 * lt + 4],
                                 sd_sb[o:o + npb, c0d:c0d + 128],
                                 d4g_sb[ck % 2][o:o + npb, f % FCH, 0:HEADS],
                                 start=False, stop=True, skip_group_check=True,
                                 tile_position=(o, 0),
                                 ).then_inc(mm, 1); C["mm"] += 1
                if lt == GST - 1 or t_i == ntiles - 1:
                    mx_free[gsp] = C["mm"]
                    al_free[gsp] = C["aa"]  # provisional; prelu below updates
                    emit_gather(gs + 2)

                if not (k8 == 7 or t_i == ntiles - 1):
                    continue

                # ---- 8-tile group: prelu, exp, mult, stair, staging ----
                g0 = t_i - k8
                nw = 4 * (k8 + 1)
                lo = 128 * gsp + 4 * (g0 - gs * GST)
                nc.scalar.wait_ge(mm, C["mm"])
                nc.scalar.wait_ge(mm, max(mw_last[s] for s in range(k8 + 1)))
                nc.scalar.activation(lr_sb[gsp][:, 0:nw], al_ps[:, lo:lo + nw],
                                     AF.Prelu, alpha=NEG_SLOPE
                                     ).then_inc(aa, 1); C["aa"] += 1
                nc.scalar.activation(
                    bass.AP(mw_sb, 128, [[8 * 132, 128], [132, k8 + 1], [1, 4]]),
                    lr_sb[gsp][:, 0:nw], AF.Exp).then_inc(aa, 1); C["aa"] += 1
                exp_done = C["aa"]
                if lt == GST - 1 or t_i == ntiles - 1:
                    al_free[gsp] = C["aa"]

                for b0 in range(g0, t_i + 1, 4):
                    b1 = min(t_i, b0 + 3)
                    nb = b1 - b0 + 1
                    s0 = b0 % 8
                    nc.vector.wait_ge(mm, C["mm"])
                    nc.vector.wait_ge(aa, exp_done)
                    with nc.allow_low_precision(reason="fp16 messages"):
                        nc.vector.tensor_tensor(
                            bass.AP(mw_sb, 132 * s0,
                                    [[8 * 132, 128], [132, nb], [32, HEADS], [1, 32]]),
                            bass.AP(he_ps, 128 * s0,
                                    [[1024, 128], [128, nb], [32, HEADS], [1, 32]]),
                            bass.AP(mw_sb, 132 * s0 + 128,
                                    [[8 * 132, 128], [132, nb], [1, HEADS], [0, 32]]),
                            OP.mult).then_inc(vv, 1); C["vv"] += 1
                    for bt in range(b0, b1 + 1):
                        he_last[bt % 8] = C["vv"]

                for bt in range(g0, t_i + 1):
                    tt = tiles[bt]
                    gff = gf0 + tt["fill"]
                    fp = gff % 2
                    if tt["fill_start"]:
                        nc.tensor.wait_ge(aa, fc_done.get(gff - 2, 0))
                    nc.tensor.wait_ge(vv, he_last[bt % 8])
                    c0 = st2_col0[tt["D"]] + 128 - tt["o"]
                    nc.tensor.matmul(
                        agg_ps[:, 256 * fp:256 * fp + 132],
                        st2_sb[:, c0:c0 + 128],
                        bass.AP(mw_sb, 132 * (bt % 8), [[8 * 132, 128], [1, 132]]),
                        start=tt["fill_start"], stop=tt["fill_end"],
                        skip_group_check=True,
                    ).then_inc(mm, 1); C["mm"] += 1
                    mw_last[bt % 8] = C["mm"]

                    if tt["fill_end"]:
                        ff = tt["fill"]
                        sgi = ff // SGF
                        sgp = sgi % 2
                        nc.scalar.wait_ge(mm, C["mm"])
                        nc.scalar.wait_ge(sc_s, stg_free[sgp])
                        nc.scalar.activation(
                            stg_sb[sgp][:, ff % SGF, 0:132],
                            agg_ps[:, 256 * fp:256 * fp + 132],
                            AF.Copy).then_inc(aa, 1); C["aa"] += 1
                        fc_done[gff] = C["aa"]
                        if ff % SGF == SGF - 1 or ff == nfills - 1:
                            nf = ff % SGF + 1
                            nc.gpsimd.wait_ge(aa, fc_done[gff])
                            if g == 0 and sgi == 0:
                                nc.gpsimd.wait_ge(cns, zero_done)
                            nc.gpsimd.dma_scatter_add(
                                bass.AP(acc_dr, g * SUBW, [[ROWW, TBL_ROWS], [1, 132]]),
                                stg_sb[sgp][:, 0:nf, :],
                                sci_sb[par][:, sgi * 128:sgi * 128 + nf * 8],
                                128 * nf, 128 * nf, 132, elem_step=ROWW,
                                single_packet=False,
                            ).then_inc(sc_s, 16); C["sc_s"] += 16
                            stg_free[sgp] = C["sc_s"]
            gfill += nfills
            d4g_free[cur_ck % 2] = C["mm"]
            if cur_ck >= 1:
                d4g_free[(cur_ck - 1) % 2] = max(d4g_free[(cur_ck - 1) % 2], 0)
            sci_free[par] = C["sc_s"]

        # ---------------- Phase C: merge + normalize ----------------
        all_sc = C["sc_s"]
        acc_free = [0, 0]   # vv counts freeing acc_sb bufs
        rd_done = [0, 0]
        ow_done = [0, 0]
        for r in range(FINR):
            p = r % 2
            nc.sync.wait_ge(sc_s, all_sc)
            nc.sync.wait_ge(vv, acc_free[p])
            nc.sync.dma_start(
                acc_sb[p][:, :, :],
                bass.AP(acc_dr, r * 1280 * ROWW,
                        [[ROWW, 128], [ROWW * 128, FINR], [1, ROWW]]),
            ).then_inc(cns, 16); C["cns"] += 16
            rd_done[p] = C["cns"]
            A = acc_sb[p]
            nc.vector.wait_ge(cns, rd_done[p])
            with nc.allow_low_precision(reason="fp16 partial sums"):
                nc.vector.tensor_tensor(t01_sb[:, :, :], A[:, :, 0:132],
                                        A[:, :, SUBW:SUBW + 132], OP.add
                                        ).then_inc(vv, 1); C["vv"] += 1
                nc.vector.tensor_tensor(t23_sb[:, :, :], A[:, :, 2 * SUBW:2 * SUBW + 132],
                                        A[:, :, 3 * SUBW:3 * SUBW + 132], OP.add
                                        ).then_inc(vv, 1); C["vv"] += 1
                nc.vector.wait_ge(vv, C["vv"])
                nc.vector.tensor_tensor(ts_sb[:, :, :], t01_sb[:, :, :],
                                        t23_sb[:, :, :], OP.add
                                        ).then_inc(vv, 1); C["vv"] += 1
            acc_free[p] = C["vv"]
            nc.vector.wait_ge(vv, C["vv"])
            nc.vector.tensor_scalar_add(zf_sb[:, :, :], ts_sb[:, :, 128:132], 1e-16
                                        ).then_inc(vv, 1); C["vv"] += 1
            nc.vector.wait_ge(vv, C["vv"])
            nc.vector.reciprocal(rz_sb[:, :, :], zf_sb[:, :, :]).then_inc(vv, 1)
            C["vv"] += 1
            nc.vector.wait_ge(vv, C["vv"])
            nc.vector.wait_ge(ow, ow_done[p])
            nc.vector.tensor_tensor(
                bass.AP(of_sb[p], 0, [[FINR * NCH, 128], [NCH, FINR], [32, HEADS], [1, 32]]),
                bass.AP(ts_sb, 0, [[FINR * 132, 128], [132, FINR], [32, HEADS], [1, 32]]),
                bass.AP(rz_sb, 0, [[FINR * HEADS, 128], [HEADS, FINR], [1, HEADS], [0, 32]]),
                OP.mult).then_inc(vv, 1); C["vv"] += 1
            nc.vector.wait_ge(vv, C["vv"])
            nc.vector.tensor_tensor(
                of_sb[p][:, :, :], of_sb[p][:, :, :],
                bass.AP(biasr_sb, 0, [[NCH, 128], [0, FINR], [1, NCH]]),
                OP.add).then_inc(vv, 1); C["vv"] += 1
            nc.sync.wait_ge(vv, C["vv"])
            nc.sync.dma_start(
                bass.AP(out_dr, r * 1280 * NCH,
                        [[NCH, 128], [NCH * 128, FINR], [1, NCH]]),
                of_sb[p][:, :, :],
            ).then_inc(ow, 16); C["ow"] += 16
            ow_done[p] = C["ow"]
        nc.sync.wait_ge(ow, C["ow"])
        nc.gpsimd.wait_ge(ow, C["ow"])
        nc.tensor.wait_ge(ow, C["ow"])

    nc.compile()
    return nc


_CACHE = {}


def kernel(x, edge_index, weight, att, bias):
    import sys
    if '/opt/trn_rl_repo' not in sys.path:
        sys.path.insert(0, '/opt/trn_rl_repo')
    from concourse.bass_utils import run_bass_kernel_spmd

    shared, per_core, meta = _host_prep(x, edge_index, weight, att, bias)
    key = "prog"
    if key not in _CACHE:
        _CACHE[key] = _build_program(meta)
    nc = _CACHE[key]
    in_maps = [dict(shared, **per_core[c]) for c in range(NCORES)]
    res = run_bass_kernel_spmd(nc, in_maps, list(range(NCORES)))
    outs = [res.results[c]["out"][:NPC, :] for c in range(NCORES)]
    return np.concatenate(outs, axis=0).astype(np.float32)
